# revision 15
# baseline (speedup 1.0000x reference)
"""CFConv (SchNet continuous-filter conv) on 8 Trainium2 NeuronCores.

Algorithm (edge-parallel, dst-sorted):
  hv = node_feats @ Wn + bn                    [V, H]
  he = ssp(ssp(edge_feats @ We1 + be1) @ We2 + be2)
  m  = hv[src] * he                            [E, H]
  h  = segment_sum(m, dst, V)                  [V, H]
  out= ssp(h @ Wo + bo)                        [V, H]
  where ssp(x) = softplus(x) - log 2 = ln(0.5 + 0.5 e^x)

Host (pure data movement / sharding):
  - sorts edges by dst, pads every node to even degree (dummy edges with
    G-row = 0 so their message is exactly 0)
  - gathers G = (node_feats + bn @ Wn^-1)[src]  (the hv[src] gather done as
    host data movement; Wn matmul stays on device)
  - packs edge/gathered-node streams feature-major, two stacked halves per
    core ([128, C]: partitions 0-63 = half A, 64-127 = half B)
  - computes per-window segment-end extraction index lists

Device (per core, SPMD):
  per 1024-col tile: 3 matmuls (block-diag bf16 weights, K=128) ->
  ACT exp/ln pairs (= exact shifted-softplus) -> DVE multiply (reads PSUM)
  -> GPSIMD pair-sum -> DVE running cumsum (tensor_tensor_scan) ->
  GPSIMD ap_gather segment-end extraction -> shifted subtract = segment
  sums -> output projection + ssp -> DMA out.
"""

import os
import shutil
import struct
import tempfile

import numpy as np
import ml_dtypes

V = 100000
E = 1600000
D = 64          # node_in = edge_in = hidden = out
TILE = 1024     # columns per device tile
WIN = 4096      # pair-columns per extraction window
USE_SSP_TABLE = os.environ.get("KERNEL_SSP", "1") == "1"

BF16 = ml_dtypes.bfloat16


def _make_ssp_act_tables():
    """Build an act-table dir where the Silu entry of silu_and_others
    computes ssp(x) = softplus(x) - log2 = ln(0.5 + 0.5 e^x).

    The bucket table is [d0,d1,d2,d3,x0,0,0,0] per 32B entry, evaluated as
    d0 + t*(d1 + t*(d2 + t*d3)) with t = x - x0 (Taylor at x0, verified
    against the stock silu entries). We keep silu's bucket partition /
    ctrl / profile structure and refit every coefficient to ssp.
    Returns the path to the patched act_info.json.
    """
    import json
    from neuronxcc.driver.Job import Job
    from neuronxcc.driver.jobs.support.FindActInfo import findActInfoFile

    src_json = findActInfoFile(Job.getPackageDir(), "gen3")
    src_dir = os.path.dirname(src_json)
    dst_dir = tempfile.mkdtemp(prefix="ssp_act_")
    for f in os.listdir(src_dir):
        shutil.copy(os.path.join(src_dir, f), os.path.join(dst_dir, f))

    prof = json.load(open(os.path.join(dst_dir, "silu_and_others.json")))
    bkt_path = os.path.join(dst_dir, prof["bkt_bin"])
    bkt = np.fromfile(bkt_path, dtype=np.float32).reshape(-1, 8).copy()

    def sig(x):
        return 1.0 / (1.0 + np.exp(-x))

    def ssp64(x):
        return (np.log1p(np.exp(-np.abs(x))) + np.maximum(x, 0.0)
                - np.log(2.0))

    # silu occupies buckets [0, 912): 0..907 normal, 908/909 small-signal
    # pos/neg, 910 large-pos, 911 large-neg.
    x0 = bkt[:908, 4].astype(np.float64)
    s = sig(x0)
    bkt[:908, 0] = ssp64(x0)
    bkt[:908, 1] = s
    bkt[:908, 2] = (s * (1 - s)) / 2.0
    bkt[:908, 3] = (s * (1 - s) * (1 - 2 * s)) / 6.0
    ln2 = float(np.log(2.0))
    bkt[908] = [0.0, 0.5, 0.125, 0.0, 0.0, 0, 0, 0]   # |x| small: taylor at 0
    bkt[909] = [0.0, 0.5, 0.125, 0.0, 0.0, 0, 0, 0]
    bkt[910] = [-ln2, 1.0, 0.0, 0.0, 0.0, 0, 0, 0]    # x >> 0: x - ln2
    bkt[911] = [-ln2, 0.0, 0.0, 0.0, 0.0, 0, 0, 0]    # x << 0: -ln2
    bkt.tofile(bkt_path)

    for ent in prof["profile_meta_data"]:
        if ent["func_name"].startswith("silu"):
            ent["fninf_result"] = struct.unpack(
                "<I", struct.pack("<f", -ln2))[0]
    json.dump(prof, open(os.path.join(dst_dir, "silu_and_others.json"), "w"))
    return os.path.join(dst_dir, "act_info.json")


def _ssp_np(x):
    return np.log1p(np.exp(-np.abs(x))) + np.maximum(x, 0.0) - np.log(2.0)


def _wrap_idx(lists, k_fix):
    """lists: 8 python/np int arrays (one per 16-partition group), each
    padded to k_fix. Returns [128, k_fix//16] int16 wrapped layout:
    index i of group g lives at [16*g + i%16, i//16]."""
    out = np.zeros((128, k_fix // 16), dtype=np.int16)
    for g in range(8):
        arr = np.asarray(lists[g], dtype=np.int16).reshape(k_fix // 16, 16)
        out[16 * g:16 * g + 16, :] = arr.T
    return out


def _host_prep(node_feats, edge_feats, src, dst, Wn, bn, We1, be1, We2, be2,
               Wo, bo):
    nfc = node_feats

    # ---- dst-sort + even-degree padding
    order = np.argsort(dst, kind="stable")
    deg = np.bincount(dst, minlength=V)
    pad = (deg % 2).astype(np.int64)
    deg_p = deg + pad
    start = np.zeros(V + 1, dtype=np.int64)
    np.cumsum(deg_p, out=start[1:])
    L = int(start[-1])
    cumpad = np.zeros(V + 1, dtype=np.int64)
    np.cumsum(pad, out=cumpad[1:])
    dst_sorted = dst[order]
    slot = np.arange(E, dtype=np.int64) + cumpad[dst_sorted]

    Ep = np.zeros((L, D), dtype=BF16)
    Ep[slot] = edge_feats[order].astype(BF16)
    Gp = np.zeros((L, D), dtype=BF16)
    Gp[slot] = nfc[src[order]].astype(BF16)

    # ---- shard into 8 cores x 2 halves at node boundaries
    bounds = [0]
    for k in range(1, 17):  # 16 half-boundaries
        tgt = L * k // 16
        n = int(np.searchsorted(start, tgt, side="left"))
        n = min(n, V)
        bounds.append(n)
    bounds[-1] = V
    hb = [(int(start[bounds[i]]), int(start[bounds[i + 1]]))
          for i in range(16)]  # slot ranges per half
    lens = [e - s for s, e in hb]
    c_col = -(-max(lens) // TILE) * TILE  # round up to TILE
    c_pair = c_col // 2
    n_w = -(-c_pair // WIN)

    # ---- extraction lists per (core, half, window)
    # node n of half H (nodes bounds[i]..bounds[i+1]) ends at pair
    # (start[n+1] - half_slot_start)//2 - 1
    ends = []       # per half: np.int64 array of end-pairs (node order)
    nodelists = []  # per half: node ids
    for i in range(16):
        nlo, nhi = bounds[i], bounds[i + 1]
        s0 = hb[i][0]
        nn = np.arange(nlo, nhi, dtype=np.int64)
        ep = (start[nn + 1] - s0) // 2 - 1
        keep = ep >= 0  # drop only a zero-degree node at the very start
        ends.append(ep[keep])
        nodelists.append(nn[keep])

    counts = np.zeros((16, n_w), dtype=np.int64)
    for i in range(16):
        w = ends[i] // WIN
        cnt = np.bincount(w, minlength=n_w)
        counts[i] = cnt
    k_fix = int(counts.max())
    k_fix = -(-k_fix // 16) * 16

    # per half: wrapped idx array [16, n_w * k_fix / 16] and column->node map
    half_idx = []
    col_node = []  # per half: int64 array len n_w*k_fix, -1 for dup/junk
    for i in range(16):
        w = ends[i] // WIN
        loc = ends[i] - w * WIN
        idx_full = np.zeros(n_w * k_fix, dtype=np.int64)
        cmap = np.full(n_w * k_fix, -1, dtype=np.int64)
        off = 0
        for ww in range(n_w):
            sel = w == ww
            k = int(counts[i, ww])
            lw = loc[sel]
            if k > 0:
                idx_full[ww * k_fix:ww * k_fix + k] = lw
                idx_full[ww * k_fix + k:(ww + 1) * k_fix] = lw[-1]
                cmap[ww * k_fix:ww * k_fix + k] = nodelists[i][off:off + k]
            # k == 0: leave zeros (junk, dropped)
            off += k
        half_idx.append(idx_full)
        col_node.append(cmap)

    # ---- per-core input arrays
    in_maps = []
    w_cat = np.zeros((128, 4 * 128), dtype=BF16)  # We1b|We2b|Wnb|Wob
    for j, Wm in enumerate((We1, We2, Wn, Wo)):
        wb = np.zeros((128, 128), dtype=np.float32)
        wb[:64, :64] = Wm
        wb[64:, 64:] = Wm
        w_cat[:, j * 128:(j + 1) * 128] = wb.astype(BF16)
    b_cat = np.zeros((128, 4), dtype=np.float32)  # be1|be2|bo|bn
    for j, bv in enumerate((be1, be2, bo, bn)):
        b_cat[:64, j] = bv
        b_cat[64:, j] = bv

    idx_cols = n_w * k_fix // 16
    for cidx in range(8):
        iA, iB = 2 * cidx, 2 * cidx + 1
        xe = np.zeros((128, c_col), dtype=BF16)
        xg = np.zeros((128, c_col), dtype=BF16)
        for half, i in ((0, iA), (1, iB)):
            s, e = hb[i]
            xe[half * 64:(half + 1) * 64, :e - s] = Ep[s:e].T
            xg[half * 64:(half + 1) * 64, :e - s] = Gp[s:e].T
        idx_arr = np.zeros((128, idx_cols), dtype=np.int16)
        for half, i in ((0, iA), (1, iB)):
            full = half_idx[i]  # [n_w * k_fix]
            wrapped = full.reshape(-1, 16).astype(np.int16)  # [n_w*k_fix/16,16]
            for g in range(4):
                idx_arr[64 * half + 16 * g: 64 * half + 16 * (g + 1), :] = wrapped.T
        in_maps.append({
            "xe": xe, "xg": xg, "idx": idx_arr,
            "wcat": w_cat, "bcat": b_cat,
        })

    meta = dict(c_col=c_col, c_pair=c_pair, n_w=n_w, k_fix=k_fix,
                col_node=col_node, bounds=bounds)
    return in_maps, meta


def _build_program(c_col, n_w, k_fix, use_ssp=False):
    import concourse.bass as bass
    import concourse.tile as tile
    from concourse import bacc, mybir

    F32 = mybir.dt.float32
    B16 = mybir.dt.bfloat16
    I16 = mybir.dt.int16
    AF = mybir.ActivationFunctionType
    OP = mybir.AluOpType

    c_pair = c_col // 2
    n_tiles = c_col // TILE
    nt_cols = n_w * k_fix
    nt_pad = -(-nt_cols // TILE) * TILE
    idx_cols = n_w * k_fix // 16

    nc = bacc.Bacc("TRN2", target_bir_lowering=False, debug=False,
                   enable_asserts=False, num_devices=8)
    i_xe = nc.dram_tensor("xe", [128, c_col], B16, kind="ExternalInput").ap()
    i_xg = nc.dram_tensor("xg", [128, c_col], B16, kind="ExternalInput").ap()
    i_idx = nc.dram_tensor("idx", [128, idx_cols], I16,
                           kind="ExternalInput").ap()
    i_w = nc.dram_tensor("wcat", [128, 512], B16, kind="ExternalInput").ap()
    i_b = nc.dram_tensor("bcat", [128, 4], F32, kind="ExternalInput").ap()
    o_out = nc.dram_tensor("out", [128, nt_pad], F32,
                           kind="ExternalOutput").ap()

    with tile.TileContext(nc) as tc:
        with (
            tc.tile_pool(name="const", bufs=1) as cpool,
            tc.tile_pool(name="stream", bufs=3) as spool,
            tc.tile_pool(name="work", bufs=2) as wpool,
            tc.tile_pool(name="psum1", bufs=2, space="PSUM") as ppool1,
            tc.tile_pool(name="psum2", bufs=1, space="PSUM") as ppool2,
            tc.tile_pool(name="psum3", bufs=1, space="PSUM") as ppool3,
            tc.tile_pool(name="swin", bufs=2) as swpool,
            tc.tile_pool(name="persist", bufs=1) as perpool,
        ):
            w = cpool.tile([128, 512], B16)
            nc.sync.dma_start(w[:], i_w[:])
            bias = cpool.tile([128, 4], F32)
            nc.sync.dma_start(bias[:], i_b[:])
            idx = cpool.tile([128, idx_cols], I16)
            nc.sync.dma_start(idx[:], i_idx[:])
            half = cpool.tile([128, 1], F32)
            nc.vector.memset(half[:], 0.5)
            zero = cpool.tile([128, 1], F32)
            nc.vector.memset(zero[:], 0.0)

            carry = perpool.tile([128, 1], F32)
            nc.vector.tensor_copy(carry[:], zero[:])
            compact = perpool.tile([128, 1 + nt_cols], F32)
            nc.vector.memset(compact[:, 0:1], 0.0)

            we1, we2, wn, wo = (w[:, 128 * j:128 * (j + 1)] for j in range(4))
            be1, be2, bo, bn_ap = (bias[:, j:j + 1] for j in range(4))

            tiles_per_win = WIN // (TILE // 2)  # scan pairs per tile = TILE/2
            s_win = None
            for t in range(n_tiles):
                woff = t % tiles_per_win  # position within current window
                if woff == 0:
                    wlen = min(WIN, c_pair - (t // tiles_per_win) * WIN)
                    s_win = swpool.tile([128, wlen], F32, tag="swin")

                xe_t = spool.tile([128, TILE], B16, tag="xe")
                nc.sync.dma_start(xe_t[:], i_xe[:, t * TILE:(t + 1) * TILE])
                xg_t = spool.tile([128, TILE], B16, tag="xg")
                nc.sync.dma_start(xg_t[:], i_xg[:, t * TILE:(t + 1) * TILE])

                z1 = ppool1.tile([128, TILE], F32, space="PSUM", tag="z1")
                for q in range(TILE // 512):
                    nc.tensor.matmul(z1[:, 512 * q:512 * (q + 1)], lhsT=we1,
                                     rhs=xe_t[:, 512 * q:512 * (q + 1)],
                                     start=True, stop=True)
                l1 = wpool.tile([128, TILE], B16, tag="l1")
                if use_ssp:
                    nc.scalar.activation(l1[:], z1[:], AF.Silu, bias=be1,
                                         scale=1.0)
                else:
                    t1 = wpool.tile([128, TILE], F32, tag="t1")
                    nc.scalar.activation(t1[:], z1[:], AF.Exp, bias=be1,
                                         scale=1.0)
                    nc.scalar.activation(l1[:], t1[:], AF.Ln, bias=half[:],
                                         scale=0.5)

                z2 = ppool2.tile([128, TILE], F32, space="PSUM", tag="z2")
                for q in range(TILE // 512):
                    nc.tensor.matmul(z2[:, 512 * q:512 * (q + 1)], lhsT=we2,
                                     rhs=l1[:, 512 * q:512 * (q + 1)],
                                     start=True, stop=True)
                l2 = wpool.tile([128, TILE], B16, tag="l2")
                if use_ssp:
                    nc.scalar.activation(l2[:], z2[:], AF.Silu, bias=be2,
                                         scale=1.0)
                else:
                    t2 = wpool.tile([128, TILE], F32, tag="t2")
                    nc.scalar.activation(t2[:], z2[:], AF.Exp, bias=be2,
                                         scale=1.0)
                    nc.scalar.activation(l2[:], t2[:], AF.Ln, bias=half[:],
                                         scale=0.5)

                z3 = ppool3.tile([128, TILE], F32, space="PSUM", tag="z3")
                for q in range(TILE // 512):
                    nc.tensor.matmul(z3[:, 512 * q:512 * (q + 1)], lhsT=wn,
                                     rhs=xg_t[:, 512 * q:512 * (q + 1)],
                                     start=True, stop=True)
                m = wpool.tile([128, TILE], B16, tag="m")
                nc.vector.scalar_tensor_tensor(
                    out=m[:], in0=z3[:], scalar=bn_ap, in1=l2[:],
                    op0=OP.add, op1=OP.mult)

                m3 = m[:].rearrange("p (j two) -> p j two", two=2)
                m2 = wpool.tile([128, TILE // 2], F32, tag="m2")
                nc.gpsimd.tensor_add(m2[:], m3[:, :, 0], m3[:, :, 1])

                po = woff * (TILE // 2)
                init = carry[:] if woff == 0 else s_win[:, po - 1:po]
                nc.vector.tensor_tensor_scan(
                    out=s_win[:, po:po + TILE // 2],
                    data0=zero[:].to_broadcast([128, TILE // 2]),
                    data1=m2[:], initial=init, op0=OP.add, op1=OP.add)

                last_in_win = (woff == tiles_per_win - 1) or (t == n_tiles - 1)
                if last_in_win:
                    wi = t // tiles_per_win
                    wlen = s_win.shape[1]
                    nc.vector.tensor_copy(carry[:], s_win[:, wlen - 1:wlen])
                    nc.gpsimd.ap_gather(
                        out_ap=compact[:, 1 + wi * k_fix:1 + (wi + 1) * k_fix]
                        .rearrange("p (k one) -> p k one", one=1),
                        in_ap=s_win[:].rearrange("p (n one) -> p n one", one=1),
                        idxs_ap=idx[:, wi * (k_fix // 16):(wi + 1) * (k_fix // 16)],
                        channels=128, num_elems=wlen, d=1, num_idxs=k_fix)

            # ---- tail: h = diff(compact), out = ssp(h @ Wo + bo)
            hbuf = perpool.tile([128, nt_pad], B16)
            if nt_pad > nt_cols:
                nc.vector.memset(hbuf[:, nt_cols:], 0.0)
            nc.vector.tensor_tensor(out=hbuf[:, :nt_cols],
                                    in0=compact[:, 1:1 + nt_cols],
                                    in1=compact[:, 0:nt_cols],
                                    op=OP.subtract)
            for t in range(nt_pad // TILE):
                z4 = ppool1.tile([128, TILE], F32, space="PSUM", tag="z1")
                for q in range(TILE // 512):
                    nc.tensor.matmul(z4[:, 512 * q:512 * (q + 1)], lhsT=wo,
                                     rhs=hbuf[:, t * TILE + 512 * q:
                                              t * TILE + 512 * (q + 1)],
                                     start=True, stop=True)
                o4 = wpool.tile([128, TILE], F32, tag="t2")
                if use_ssp:
                    nc.scalar.activation(o4[:], z4[:], AF.Silu, bias=bo,
                                         scale=1.0)
                else:
                    t4 = wpool.tile([128, TILE], F32, tag="t1")
                    nc.scalar.activation(t4[:], z4[:], AF.Exp, bias=bo,
                                         scale=1.0)
                    nc.scalar.activation(o4[:], t4[:], AF.Ln, bias=half[:],
                                         scale=0.5)
                nc.sync.dma_start(o_out[:, t * TILE:(t + 1) * TILE], o4[:])

    nc.compile()
    return nc


_CACHE = {}


def kernel(node_feats, edge_feats, src, dst, Wn, bn, We1, be1, We2, be2,
           Wo, bo, _collect_perf=None):
    from concourse.bass_utils import run_bass_kernel_spmd

    in_maps, meta = _host_prep(node_feats, edge_feats, src, dst, Wn, bn,
                               We1, be1, We2, be2, Wo, bo)
    if USE_SSP_TABLE and "BASS_ACT_ROOT_JSON_PATH" not in os.environ:
        os.environ["BASS_ACT_ROOT_JSON_PATH"] = _make_ssp_act_tables()
    key = (meta["c_col"], meta["n_w"], meta["k_fix"], USE_SSP_TABLE)
    if key not in _CACHE:
        _CACHE[key] = _build_program(*key)
    nc = _CACHE[key]

    res = run_bass_kernel_spmd(nc, in_maps, core_ids=list(range(8)),
                               **(_collect_perf or {}))
    if _collect_perf is not None:
        _collect_perf["result"] = res

    return _assemble([res.results[cidx]["out"] for cidx in range(8)], meta,
                     bo)


def _assemble(outs, meta, bo):
    out = np.empty((V, D), dtype=np.float32)
    out[:] = _ssp_np(bo)[None, :]  # zero-degree nodes dropped from lists
    nt_cols = meta["n_w"] * meta["k_fix"]
    for cidx in range(8):
        o = outs[cidx]  # [128, nt_pad] fp32
        for half in (0, 1):
            cmap = meta["col_node"][2 * cidx + half]
            valid = cmap >= 0
            nodes = cmap[valid]
            out[nodes] = o[64 * half:64 * (half + 1), :nt_cols][:, valid].T
    return out


# revision 16
# speedup vs baseline: 9.8594x; 9.8594x over previous
"""CFConv (SchNet continuous-filter conv) on 8 Trainium2 NeuronCores.

Algorithm (edge-parallel, dst-sorted):
  hv = node_feats @ Wn + bn                    [V, H]
  he = ssp(ssp(edge_feats @ We1 + be1) @ We2 + be2)
  m  = hv[src] * he                            [E, H]
  h  = segment_sum(m, dst, V)                  [V, H]
  out= ssp(h @ Wo + bo)                        [V, H]
  where ssp(x) = softplus(x) - log 2 = ln(0.5 + 0.5 e^x)

Host (pure data movement / sharding):
  - sorts edges by dst, pads every node to even degree (dummy edges with
    G-row = 0 so their message is exactly 0)
  - gathers G = (node_feats + bn @ Wn^-1)[src]  (the hv[src] gather done as
    host data movement; Wn matmul stays on device)
  - packs edge/gathered-node streams feature-major, two stacked halves per
    core ([128, C]: partitions 0-63 = half A, 64-127 = half B)
  - computes per-window segment-end extraction index lists

Device (per core, SPMD):
  per 1024-col tile: 3 matmuls (block-diag bf16 weights, K=128) ->
  ACT exp/ln pairs (= exact shifted-softplus) -> DVE multiply (reads PSUM)
  -> GPSIMD pair-sum -> DVE running cumsum (tensor_tensor_scan) ->
  GPSIMD ap_gather segment-end extraction -> shifted subtract = segment
  sums -> output projection + ssp -> DMA out.
"""

import os
import shutil
import struct
import tempfile

import numpy as np
import ml_dtypes

V = 100000
E = 1600000
D = 64          # node_in = edge_in = hidden = out
TILE = 1024     # columns per device tile
WIN = 4096      # pair-columns per extraction window
USE_SSP_TABLE = os.environ.get("KERNEL_SSP", "1") == "1"

BF16 = ml_dtypes.bfloat16


def _make_ssp_act_tables():
    """Build an act-table dir where the Silu entry of silu_and_others
    computes ssp(x) = softplus(x) - log2 = ln(0.5 + 0.5 e^x).

    The bucket table is [d0,d1,d2,d3,x0,0,0,0] per 32B entry, evaluated as
    d0 + t*(d1 + t*(d2 + t*d3)) with t = x - x0 (Taylor at x0, verified
    against the stock silu entries). We keep silu's bucket partition /
    ctrl / profile structure and refit every coefficient to ssp.
    Returns the path to the patched act_info.json.
    """
    import json
    from neuronxcc.driver.Job import Job
    from neuronxcc.driver.jobs.support.FindActInfo import findActInfoFile

    src_json = findActInfoFile(Job.getPackageDir(), "gen3")
    src_dir = os.path.dirname(src_json)
    dst_dir = tempfile.mkdtemp(prefix="ssp_act_")
    for f in os.listdir(src_dir):
        shutil.copy(os.path.join(src_dir, f), os.path.join(dst_dir, f))

    prof = json.load(open(os.path.join(dst_dir, "silu_and_others.json")))
    bkt_path = os.path.join(dst_dir, prof["bkt_bin"])
    bkt = np.fromfile(bkt_path, dtype=np.float32).reshape(-1, 8).copy()

    def sig(x):
        return 1.0 / (1.0 + np.exp(-x))

    def ssp64(x):
        return (np.log1p(np.exp(-np.abs(x))) + np.maximum(x, 0.0)
                - np.log(2.0))

    # silu occupies buckets [0, 912): 0..907 normal, 908/909 small-signal
    # pos/neg, 910 large-pos, 911 large-neg.
    x0 = bkt[:908, 4].astype(np.float64)
    s = sig(x0)
    bkt[:908, 0] = ssp64(x0)
    bkt[:908, 1] = s
    bkt[:908, 2] = (s * (1 - s)) / 2.0
    bkt[:908, 3] = (s * (1 - s) * (1 - 2 * s)) / 6.0
    ln2 = float(np.log(2.0))
    bkt[908] = [0.0, 0.5, 0.125, 0.0, 0.0, 0, 0, 0]   # |x| small: taylor at 0
    bkt[909] = [0.0, 0.5, 0.125, 0.0, 0.0, 0, 0, 0]
    bkt[910] = [-ln2, 1.0, 0.0, 0.0, 0.0, 0, 0, 0]    # x >> 0: x - ln2
    bkt[911] = [-ln2, 0.0, 0.0, 0.0, 0.0, 0, 0, 0]    # x << 0: -ln2
    bkt.tofile(bkt_path)

    for ent in prof["profile_meta_data"]:
        if ent["func_name"].startswith("silu"):
            ent["fninf_result"] = struct.unpack(
                "<I", struct.pack("<f", -ln2))[0]
    json.dump(prof, open(os.path.join(dst_dir, "silu_and_others.json"), "w"))
    return os.path.join(dst_dir, "act_info.json")


def _ssp_np(x):
    return np.log1p(np.exp(-np.abs(x))) + np.maximum(x, 0.0) - np.log(2.0)


def _wrap_idx(lists, k_fix):
    """lists: 8 python/np int arrays (one per 16-partition group), each
    padded to k_fix. Returns [128, k_fix//16] int16 wrapped layout:
    index i of group g lives at [16*g + i%16, i//16]."""
    out = np.zeros((128, k_fix // 16), dtype=np.int16)
    for g in range(8):
        arr = np.asarray(lists[g], dtype=np.int16).reshape(k_fix // 16, 16)
        out[16 * g:16 * g + 16, :] = arr.T
    return out


def _host_prep(node_feats, edge_feats, src, dst, Wn, bn, We1, be1, We2, be2,
               Wo, bo):
    nfc = node_feats

    # ---- dst-sort + even-degree padding
    order = np.argsort(dst, kind="stable")
    deg = np.bincount(dst, minlength=V)
    pad = (deg % 2).astype(np.int64)
    deg_p = deg + pad
    start = np.zeros(V + 1, dtype=np.int64)
    np.cumsum(deg_p, out=start[1:])
    L = int(start[-1])
    cumpad = np.zeros(V + 1, dtype=np.int64)
    np.cumsum(pad, out=cumpad[1:])
    dst_sorted = dst[order]
    slot = np.arange(E, dtype=np.int64) + cumpad[dst_sorted]

    Ep = np.zeros((L, D), dtype=BF16)
    Ep[slot] = edge_feats[order].astype(BF16)
    Gp = np.zeros((L, D), dtype=BF16)
    Gp[slot] = nfc[src[order]].astype(BF16)

    # ---- shard into 8 cores x 2 halves at node boundaries
    bounds = [0]
    for k in range(1, 17):  # 16 half-boundaries
        tgt = L * k // 16
        n = int(np.searchsorted(start, tgt, side="left"))
        n = min(n, V)
        bounds.append(n)
    bounds[-1] = V
    hb = [(int(start[bounds[i]]), int(start[bounds[i + 1]]))
          for i in range(16)]  # slot ranges per half
    lens = [e - s for s, e in hb]
    c_col = -(-max(lens) // (2 * TILE)) * (2 * TILE)  # round to 2*TILE
    c_pair = c_col // 2
    n_w = -(-c_pair // WIN)

    # ---- extraction lists per (core, half, window)
    # node n of half H (nodes bounds[i]..bounds[i+1]) ends at pair
    # (start[n+1] - half_slot_start)//2 - 1
    ends = []       # per half: np.int64 array of end-pairs (node order)
    nodelists = []  # per half: node ids
    for i in range(16):
        nlo, nhi = bounds[i], bounds[i + 1]
        s0 = hb[i][0]
        nn = np.arange(nlo, nhi, dtype=np.int64)
        ep = (start[nn + 1] - s0) // 2 - 1
        keep = ep >= 0  # drop only a zero-degree node at the very start
        ends.append(ep[keep])
        nodelists.append(nn[keep])

    counts = np.zeros((16, n_w), dtype=np.int64)
    for i in range(16):
        w = ends[i] // WIN
        cnt = np.bincount(w, minlength=n_w)
        counts[i] = cnt
    k_fix = int(counts.max())
    k_fix = -(-k_fix // 16) * 16

    # per half: wrapped idx array [16, n_w * k_fix / 16] and column->node map
    half_idx = []
    col_node = []  # per half: int64 array len n_w*k_fix, -1 for dup/junk
    for i in range(16):
        w = ends[i] // WIN
        loc = ends[i] - w * WIN
        idx_full = np.zeros(n_w * k_fix, dtype=np.int64)
        cmap = np.full(n_w * k_fix, -1, dtype=np.int64)
        off = 0
        for ww in range(n_w):
            sel = w == ww
            k = int(counts[i, ww])
            lw = loc[sel]
            if k > 0:
                idx_full[ww * k_fix:ww * k_fix + k] = lw
                idx_full[ww * k_fix + k:(ww + 1) * k_fix] = lw[-1]
                cmap[ww * k_fix:ww * k_fix + k] = nodelists[i][off:off + k]
            # k == 0: leave zeros (junk, dropped)
            off += k
        half_idx.append(idx_full)
        col_node.append(cmap)

    # ---- per-core input arrays
    in_maps = []
    w_cat = np.zeros((128, 4 * 128), dtype=BF16)  # We1b|We2b|Wnb|Wob
    for j, Wm in enumerate((We1, We2, Wn, Wo)):
        wb = np.zeros((128, 128), dtype=np.float32)
        wb[:64, :64] = Wm
        wb[64:, 64:] = Wm
        w_cat[:, j * 128:(j + 1) * 128] = wb.astype(BF16)
    b_cat = np.zeros((128, 4), dtype=np.float32)  # be1|be2|bo|bn
    for j, bv in enumerate((be1, be2, bo, bn)):
        b_cat[:64, j] = bv
        b_cat[64:, j] = bv

    idx_cols = n_w * k_fix // 16
    for cidx in range(8):
        iA, iB = 2 * cidx, 2 * cidx + 1
        xe = np.zeros((128, c_col), dtype=BF16)
        xg = np.zeros((128, c_col), dtype=BF16)
        for half, i in ((0, iA), (1, iB)):
            s, e = hb[i]
            xe[half * 64:(half + 1) * 64, :e - s] = Ep[s:e].T
            xg[half * 64:(half + 1) * 64, :e - s] = Gp[s:e].T
        idx_arr = np.zeros((128, idx_cols), dtype=np.int16)
        for half, i in ((0, iA), (1, iB)):
            full = half_idx[i]  # [n_w * k_fix]
            wrapped = full.reshape(-1, 16).astype(np.int16)  # [n_w*k_fix/16,16]
            for g in range(4):
                idx_arr[64 * half + 16 * g: 64 * half + 16 * (g + 1), :] = wrapped.T
        in_maps.append({
            "xe": xe, "xg": xg, "idx": idx_arr,
            "wcat": w_cat, "bcat": b_cat,
        })

    meta = dict(c_col=c_col, c_pair=c_pair, n_w=n_w, k_fix=k_fix,
                col_node=col_node, bounds=bounds)
    return in_maps, meta


def _build_program(c_col, n_w, k_fix, use_ssp=False):
    import concourse.bass as bass
    import concourse.tile as tile
    from concourse import bacc, mybir

    F32 = mybir.dt.float32
    B16 = mybir.dt.bfloat16
    I16 = mybir.dt.int16
    AF = mybir.ActivationFunctionType
    OP = mybir.AluOpType

    c_pair = c_col // 2
    n_tiles = c_col // TILE
    nt_cols = n_w * k_fix
    nt_pad = -(-nt_cols // TILE) * TILE
    idx_cols = n_w * k_fix // 16

    nc = bacc.Bacc("TRN2", target_bir_lowering=False, debug=False,
                   enable_asserts=False, num_devices=8)
    i_xe = nc.dram_tensor("xe", [128, c_col], B16, kind="ExternalInput").ap()
    i_xg = nc.dram_tensor("xg", [128, c_col], B16, kind="ExternalInput").ap()
    i_idx = nc.dram_tensor("idx", [128, idx_cols], I16,
                           kind="ExternalInput").ap()
    i_w = nc.dram_tensor("wcat", [128, 512], B16, kind="ExternalInput").ap()
    i_b = nc.dram_tensor("bcat", [128, 4], F32, kind="ExternalInput").ap()
    o_out = nc.dram_tensor("out", [128, nt_pad], F32,
                           kind="ExternalOutput").ap()

    with tile.TileContext(nc) as tc:
        with (
            tc.tile_pool(name="const", bufs=1) as cpool,
            tc.tile_pool(name="stream", bufs=3) as spool,
            tc.tile_pool(name="work", bufs=2) as wpool,
            tc.tile_pool(name="psum1", bufs=2, space="PSUM") as ppool1,
            tc.tile_pool(name="psum2", bufs=1, space="PSUM") as ppool2,
            tc.tile_pool(name="psum3", bufs=1, space="PSUM") as ppool3,
            tc.tile_pool(name="swin", bufs=2) as swpool,
            tc.tile_pool(name="persist", bufs=1) as perpool,
        ):
            w = cpool.tile([128, 512], B16)
            nc.sync.dma_start(w[:], i_w[:])
            bias = cpool.tile([128, 4], F32)
            nc.sync.dma_start(bias[:], i_b[:])
            idx = cpool.tile([128, idx_cols], I16)
            nc.sync.dma_start(idx[:], i_idx[:])
            half = cpool.tile([128, 1], F32)
            nc.vector.memset(half[:], 0.5)
            zero = cpool.tile([128, 1], F32)
            nc.vector.memset(zero[:], 0.0)

            carry = perpool.tile([128, 1], F32)
            nc.vector.tensor_copy(carry[:], zero[:])
            compact = perpool.tile([128, 1 + nt_cols], F32)
            nc.vector.memset(compact[:, 0:1], 0.0)

            we1, we2, wn, wo = (w[:, 128 * j:128 * (j + 1)] for j in range(4))
            be1, be2, bo, bn_ap = (bias[:, j:j + 1] for j in range(4))

            tiles_per_win = WIN // (TILE // 2)  # scan pairs per tile = TILE/2
            s_win = None
            for t in range(n_tiles):
                woff = t % tiles_per_win  # position within current window
                if woff == 0:
                    wlen = min(WIN, c_pair - (t // tiles_per_win) * WIN)
                    s_win = swpool.tile([128, wlen], F32, tag="swin")

                xe_t = spool.tile([128, TILE], B16, tag="xe")
                nc.sync.dma_start(xe_t[:], i_xe[:, t * TILE:(t + 1) * TILE])
                xg_t = spool.tile([128, TILE], B16, tag="xg")
                nc.sync.dma_start(xg_t[:], i_xg[:, t * TILE:(t + 1) * TILE])

                z1 = ppool1.tile([128, TILE], F32, space="PSUM", tag="z1")
                for q in range(TILE // 512):
                    nc.tensor.matmul(z1[:, 512 * q:512 * (q + 1)], lhsT=we1,
                                     rhs=xe_t[:, 512 * q:512 * (q + 1)],
                                     start=True, stop=True)
                l1 = wpool.tile([128, TILE], B16, tag="l1")
                if use_ssp:
                    nc.scalar.activation(l1[:], z1[:], AF.Silu, bias=be1,
                                         scale=1.0)
                else:
                    t1 = wpool.tile([128, TILE], F32, tag="t1")
                    nc.scalar.activation(t1[:], z1[:], AF.Exp, bias=be1,
                                         scale=1.0)
                    nc.scalar.activation(l1[:], t1[:], AF.Ln, bias=half[:],
                                         scale=0.5)

                z2 = ppool2.tile([128, TILE], F32, space="PSUM", tag="z2")
                for q in range(TILE // 512):
                    nc.tensor.matmul(z2[:, 512 * q:512 * (q + 1)], lhsT=we2,
                                     rhs=l1[:, 512 * q:512 * (q + 1)],
                                     start=True, stop=True)
                l2 = wpool.tile([128, TILE], B16, tag="l2")
                if use_ssp:
                    nc.scalar.activation(l2[:], z2[:], AF.Silu, bias=be2,
                                         scale=1.0)
                else:
                    t2 = wpool.tile([128, TILE], F32, tag="t2")
                    nc.scalar.activation(t2[:], z2[:], AF.Exp, bias=be2,
                                         scale=1.0)
                    nc.scalar.activation(l2[:], t2[:], AF.Ln, bias=half[:],
                                         scale=0.5)

                z3 = ppool3.tile([128, TILE], F32, space="PSUM", tag="z3")
                for q in range(TILE // 512):
                    nc.tensor.matmul(z3[:, 512 * q:512 * (q + 1)], lhsT=wn,
                                     rhs=xg_t[:, 512 * q:512 * (q + 1)],
                                     start=True, stop=True)
                if t % 2 == 0:
                    m = wpool.tile([128, 2 * TILE], B16, tag="m")
                nc.vector.scalar_tensor_tensor(
                    out=m[:, (t % 2) * TILE:(t % 2 + 1) * TILE],
                    in0=z3[:], scalar=bn_ap, in1=l2[:],
                    op0=OP.add, op1=OP.mult)

                if t % 2 == 1:
                    m3 = m[:].rearrange("p (j two) -> p j two", two=2)
                    m2 = wpool.tile([128, TILE], F32, tag="m2")
                    nc.gpsimd.tensor_add(m2[:], m3[:, :, 0], m3[:, :, 1])

                    po = (woff - 1) * (TILE // 2)
                    init = carry[:] if po == 0 else s_win[:, po - 1:po]
                    nc.vector.tensor_tensor_scan(
                        out=s_win[:, po:po + TILE],
                        data0=zero[:].to_broadcast([128, TILE]),
                        data1=m2[:], initial=init, op0=OP.add, op1=OP.add)

                last_in_win = (woff == tiles_per_win - 1) or (t == n_tiles - 1)
                if last_in_win:
                    wi = t // tiles_per_win
                    wlen = s_win.shape[1]
                    nc.vector.tensor_copy(carry[:], s_win[:, wlen - 1:wlen])
                    nc.gpsimd.ap_gather(
                        out_ap=compact[:, 1 + wi * k_fix:1 + (wi + 1) * k_fix]
                        .rearrange("p (k one) -> p k one", one=1),
                        in_ap=s_win[:].rearrange("p (n one) -> p n one", one=1),
                        idxs_ap=idx[:, wi * (k_fix // 16):(wi + 1) * (k_fix // 16)],
                        channels=128, num_elems=wlen, d=1, num_idxs=k_fix)

            # ---- tail: h = diff(compact), out = ssp(h @ Wo + bo)
            hbuf = perpool.tile([128, nt_pad], B16)
            if nt_pad > nt_cols:
                nc.vector.memset(hbuf[:, nt_cols:], 0.0)
            nc.vector.tensor_tensor(out=hbuf[:, :nt_cols],
                                    in0=compact[:, 1:1 + nt_cols],
                                    in1=compact[:, 0:nt_cols],
                                    op=OP.subtract)
            for t in range(nt_pad // TILE):
                z4 = ppool1.tile([128, TILE], F32, space="PSUM", tag="z1")
                for q in range(TILE // 512):
                    nc.tensor.matmul(z4[:, 512 * q:512 * (q + 1)], lhsT=wo,
                                     rhs=hbuf[:, t * TILE + 512 * q:
                                              t * TILE + 512 * (q + 1)],
                                     start=True, stop=True)
                o4 = wpool.tile([128, TILE], F32, tag="t2")
                if use_ssp:
                    nc.scalar.activation(o4[:], z4[:], AF.Silu, bias=bo,
                                         scale=1.0)
                else:
                    t4 = wpool.tile([128, TILE], F32, tag="t1")
                    nc.scalar.activation(t4[:], z4[:], AF.Exp, bias=bo,
                                         scale=1.0)
                    nc.scalar.activation(o4[:], t4[:], AF.Ln, bias=half[:],
                                         scale=0.5)
                nc.sync.dma_start(o_out[:, t * TILE:(t + 1) * TILE], o4[:])

    nc.compile()
    return nc


_CACHE = {}


def kernel(node_feats, edge_feats, src, dst, Wn, bn, We1, be1, We2, be2,
           Wo, bo, _collect_perf=None):
    from concourse.bass_utils import run_bass_kernel_spmd

    in_maps, meta = _host_prep(node_feats, edge_feats, src, dst, Wn, bn,
                               We1, be1, We2, be2, Wo, bo)
    if USE_SSP_TABLE and "BASS_ACT_ROOT_JSON_PATH" not in os.environ:
        os.environ["BASS_ACT_ROOT_JSON_PATH"] = _make_ssp_act_tables()
    key = (meta["c_col"], meta["n_w"], meta["k_fix"], USE_SSP_TABLE)
    if key not in _CACHE:
        _CACHE[key] = _build_program(*key)
    nc = _CACHE[key]

    res = run_bass_kernel_spmd(nc, in_maps, core_ids=list(range(8)),
                               **(_collect_perf or {}))
    if _collect_perf is not None:
        _collect_perf["result"] = res

    return _assemble([res.results[cidx]["out"] for cidx in range(8)], meta,
                     bo)


def _assemble(outs, meta, bo):
    out = np.empty((V, D), dtype=np.float32)
    out[:] = _ssp_np(bo)[None, :]  # zero-degree nodes dropped from lists
    nt_cols = meta["n_w"] * meta["k_fix"]
    for cidx in range(8):
        o = outs[cidx]  # [128, nt_pad] fp32
        for half in (0, 1):
            cmap = meta["col_node"][2 * cidx + half]
            valid = cmap >= 0
            nodes = cmap[valid]
            out[nodes] = o[64 * half:64 * (half + 1), :nt_cols][:, valid].T
    return out
